# revision 28
# baseline (speedup 1.0000x reference)
"""Bahdanau attention TRN2 kernel.

Problem shapes (hardcoded):
    encoder_outputs: [64, 2048, 512] f32
    decoder_hidden:  [64, 512] f32
    W1, W2: [512, 512]; b1, b2: [512]; V: [512]; bv: [1]
Returns (context [64, 512] f32, attn [64, 2048] f32), matching the reference.

Sharding: data-parallel over batch across 8 NeuronCores (8 batches/core).

Per-core algorithm (per batch b):
    scores = tanh(enc @ W1.T + qb) @ V        (qb = b1 + b2 + dec @ W2.T, host-computed;
                                               bv dropped - softmax is shift-invariant)
    attn   = softmax(scores)                  (no max-subtract: |scores| <= sum|V| ~ 12)
    context= attn @ enc

Device layouts:
    enc_t  [h, s] bf16  - transposed enc (host prep), moving operand of the projection matmul
    enc_g  [p, j*512+h] bf16 - enc rows grouped s = p*16 + j, rhs of the context matmul
    scores are produced as PSUM [1, 512] rows at partitions {0,32,64,96} (PE col-tiling),
    reshaped to [128, 16] (s = p*16 + j) via small SBUF->SBUF DMAs.
"""

import numpy as np
import ml_dtypes

import concourse.bass as bass
import concourse.tile as tile
from concourse import bacc, bass_isa, mybir
from concourse._compat import with_exitstack

BF16 = mybir.dt.bfloat16
F32 = mybir.dt.float32
NPBF16 = ml_dtypes.bfloat16

B, S, H = 64, 2048, 512
NCORES = 8
BPC = B // NCORES          # batches per core
ST = 4                     # "supertiles" per batch: 512 s-rows each (vdot col-tile groups)
NC_CHUNK = H // 128        # 4 chunks of 128 along h / o
NJ = S // 128              # 16 s-groups of 128 for the context matmul

AFT = mybir.ActivationFunctionType


@with_exitstack
def _bahdanau_tile_kernel(ctx, tc, enc_g, enc_t, w1t, vvec, qbt, attn_out, ctx_out):
    nc = tc.nc

    const = ctx.enter_context(tc.tile_pool(name="const", bufs=1))
    encn_pool = ctx.enter_context(tc.tile_pool(name="encn", bufs=3))
    enct_pool = ctx.enter_context(tc.tile_pool(name="enct", bufs=8))
    energy_pool = ctx.enter_context(tc.tile_pool(name="energy", bufs=18))
    sm_pool = ctx.enter_context(tc.tile_pool(name="sm", bufs=2))
    pe_psum = ctx.enter_context(tc.tile_pool(name="pe_ps", bufs=2, space="PSUM"))
    sc_psum = ctx.enter_context(tc.tile_pool(name="sc_ps", bufs=2, space="PSUM"))
    small_psum = ctx.enter_context(tc.tile_pool(name="small_ps", bufs=2, space="PSUM"))

    # --- constants ---
    w1t_sb = []   # [128, 512] bf16 per h-chunk, cols = o
    vv_sb = []    # [128, 1] bf16 per o-chunk
    qbt_sb = []   # [128, BPC] f32 per o-chunk
    for c in range(NC_CHUNK):
        wt = const.tile([128, H], BF16, name=f"w1t{c}", tag=f"w1t{c}")
        nc.scalar.dma_start(wt[:], w1t[c * 128:(c + 1) * 128, :])
        w1t_sb.append(wt)
        vv = const.tile([128, 32], BF16, name=f"vv{c}", tag=f"vv{c}")
        nc.scalar.dma_start(vv[:], vvec[c * 128:(c + 1) * 128, :])
        vv_sb.append(vv)
        qb = const.tile([128, BPC], F32, name=f"qbt{c}", tag=f"qbt{c}")
        nc.scalar.dma_start(qb[:], qbt[c * 128:(c + 1) * 128, :])
        qbt_sb.append(qb)


    state = {}  # per-batch tiles handed from one pipeline stage to the next

    def emit_loads(b):
        enct = []
        for c in range(NC_CHUNK):
            et = enct_pool.tile([128, S], BF16, name="enct", tag="enct")
            nc.sync.dma_start(et[:], enc_t[b * H + c * 128:b * H + (c + 1) * 128, :])
            enct.append(et)
        encn = encn_pool.tile([128, NJ * H], BF16, name="encn", tag="encn")
        nc.sync.dma_start(encn[:], enc_g[b * 128:(b + 1) * 128, :])
        state[b] = {"enct": enct, "encn": encn}

    def emit_mm1(b):
        st_ = state[b]
        enct = st_["enct"]
        energies = {}
        for stp in range(ST // 2):
            for oc in range(NC_CHUNK):
                pe = pe_psum.tile([128, 1024], F32, name="pe", tag="pe")
                for st2 in range(2):
                    for hc in range(NC_CHUNK):
                        nc.tensor.matmul(
                            pe[:, st2 * 512:(st2 + 1) * 512],
                            lhsT=w1t_sb[hc][:, oc * 128:(oc + 1) * 128],
                            rhs=enct[hc][:, stp * 1024 + st2 * 512:stp * 1024 + (st2 + 1) * 512],
                            start=(hc == 0),
                            stop=(hc == NC_CHUNK - 1),
                        )
                en = energy_pool.tile([128, 1024], BF16, name="en", tag="en")
                nc.scalar.activation(en[:], pe[:], AFT.Tanh, bias=qbt_sb[oc][:, b:b + 1])
                energies[(stp, oc)] = en
        st_["energies"] = energies

    def emit_vdot(b):
        st_ = state[b]
        energies = st_["energies"]
        # V-dot: scores[st*512 + n] at psum partition 32*st (col groups)
        sc = sc_psum.tile([128, 512], F32, name="sc", tag="sc")
        for oc in range(NC_CHUNK):
            for st in range(ST):
                nc.tensor.matmul(
                    sc[32 * st:32 * st + 32, :],
                    lhsT=vv_sb[oc][:],
                    rhs=energies[(st // 2, oc)][:, (st % 2) * 512:(st % 2 + 1) * 512],
                    start=(oc == 0),
                    stop=(oc == NC_CHUNK - 1),
                    tile_position=(0, 32 * st),
                    skip_group_check=True,
                )
        # drain the whole psum tile in one parallel-partition copy (rows
        # outside {0,32,64,96} are garbage and never read), then reshape
        # to [128, 16] (s = p*16 + j) with one SBUF->SBUF DMA
        sc4 = sm_pool.tile([128, 512], F32, name="sc4", tag="sc4")
        nc.vector.tensor_copy(sc4[:], sc[:])
        scT = sm_pool.tile([128, NJ], F32, name="scT", tag="scT")
        nc.scalar.dma_start(scT[:], sc4[0:128:32, :])
        st_["scT"] = scT

    def emit_softmax(b):
        st_ = state[b]
        scT = st_["scT"]
        # softmax (unnormalized exp; fold 1/sum in at the end)
        expT = sm_pool.tile([128, NJ], F32, name="expT", tag="expT")
        nc.scalar.activation(expT[:], scT[:], AFT.Exp)
        expTb = sm_pool.tile([128, NJ], BF16, name="expTb", tag="expTb")
        nc.vector.tensor_copy(expTb[:], expT[:])

        expP = sm_pool.tile([128, 1], F32, name="expP", tag="expP")
        nc.vector.tensor_reduce(
            expP[:], expT[:], axis=mybir.AxisListType.X, op=mybir.AluOpType.add
        )
        sum_bc = sm_pool.tile([128, 1], F32, name="sum_bc", tag="sum_bc")
        nc.gpsimd.partition_all_reduce(
            sum_bc[:], expP[:], channels=128, reduce_op=bass_isa.ReduceOp.add
        )
        recip_bc = sm_pool.tile([128, 1], F32, name="recip_bc", tag="recip_bc")
        nc.vector.reciprocal(recip_bc[:], sum_bc[:])
        st_.update(expT=expT, expTb=expTb, recip_bc=recip_bc)

    def emit_mm2_outputs(b):
        st_ = state[b]
        encn, expT, expTb, recip_bc = st_["encn"], st_["expT"], st_["expTb"], st_["recip_bc"]
        ps_ctx = small_psum.tile([1, 512], F32, name="ps_ctx", tag="small")
        for j in range(NJ):
            nc.tensor.matmul(
                ps_ctx[:],
                lhsT=expTb[:, j:j + 1],
                rhs=encn[:, j * 512:(j + 1) * 512],
                start=(j == 0),
                stop=(j == NJ - 1),
            )
        ctx_sb = sm_pool.tile([1, 512], F32, name="ctx_sb", tag="ctx_sb")
        nc.scalar.activation(ctx_sb[:], ps_ctx[:], AFT.Copy, scale=recip_bc[0:1, :])
        nc.gpsimd.dma_start(ctx_out[b:b + 1, :], ctx_sb[:])

        attn_sb = sm_pool.tile([128, NJ], F32, name="attn_sb", tag="attn_sb")
        nc.vector.tensor_scalar_mul(attn_sb[:], expT[:], recip_bc[:])
        nc.gpsimd.dma_start(attn_out[b * 128:(b + 1) * 128, :], attn_sb[:])
        del state[b]

    # Software pipeline: PE stream per batch is
    #   ... vdot(b-1), mm1(b), mm2(b-1), vdot(b), mm1(b+1), ...
    # The exp of b-1 is emitted AFTER tanh(b) on the ACT queue so the
    # reshape-DMA wait never head-of-line-blocks the tanh stream.
    for b in range(BPC):
        emit_loads(b)
        if b >= 1:
            emit_vdot(b - 1)
        emit_mm1(b)
        if b >= 1:
            emit_softmax(b - 1)
            emit_mm2_outputs(b - 1)
    emit_vdot(BPC - 1)
    emit_softmax(BPC - 1)
    emit_mm2_outputs(BPC - 1)


def build_program():
    nc = bacc.Bacc(
        "TRN2",
        target_bir_lowering=False,
        debug=False,
        enable_asserts=False,
        num_devices=NCORES,
    )
    enc_g = nc.dram_tensor("enc_g", [BPC * 128, NJ * H], BF16, kind="ExternalInput").ap()
    enc_t = nc.dram_tensor("enc_t", [BPC * H, S], BF16, kind="ExternalInput").ap()
    w1t = nc.dram_tensor("w1t", [H, H], BF16, kind="ExternalInput").ap()
    vvec = nc.dram_tensor("vvec", [H, 32], BF16, kind="ExternalInput").ap()
    qbt = nc.dram_tensor("qbt", [H, BPC], F32, kind="ExternalInput").ap()
    attn_out = nc.dram_tensor("attn_out", [BPC * 128, NJ], F32, kind="ExternalOutput").ap()
    ctx_out = nc.dram_tensor("ctx_out", [BPC, H], F32, kind="ExternalOutput").ap()

    with tile.TileContext(nc) as tc:
        _bahdanau_tile_kernel(tc, enc_g, enc_t, w1t, vvec, qbt, attn_out, ctx_out)
    nc.compile()
    return nc


def prepare_in_maps(encoder_outputs, decoder_hidden, W1, b1, W2, b2, V, bv):
    enc = np.asarray(encoder_outputs, dtype=np.float32)
    dec = np.asarray(decoder_hidden, dtype=np.float32)
    W1 = np.asarray(W1, dtype=np.float32)
    W2 = np.asarray(W2, dtype=np.float32)
    b1 = np.asarray(b1, dtype=np.float32)
    b2 = np.asarray(b2, dtype=np.float32)
    V = np.asarray(V, dtype=np.float32)

    enc_bf = enc.astype(NPBF16)
    # s = p*16 + j grouping: [B, 128, 16, H] -> per core [BPC*128, 16*H]
    enc_g_all = enc_bf.reshape(B, 128, NJ, H)
    enc_t_all = np.ascontiguousarray(enc_bf.transpose(0, 2, 1))  # [B, H, S]

    qb_all = (b1 + b2 + dec @ W2.T).astype(np.float32)  # [B, H]
    w1t_np = np.ascontiguousarray(W1.T).astype(NPBF16)  # [h, o]
    vvec_np = np.ascontiguousarray(np.tile(V.astype(NPBF16).reshape(H, 1), (1, 32)))

    in_maps = []
    for c in range(NCORES):
        bs = slice(c * BPC, (c + 1) * BPC)
        in_maps.append({
            "enc_g": np.ascontiguousarray(enc_g_all[bs]).reshape(BPC * 128, NJ * H),
            "enc_t": np.ascontiguousarray(enc_t_all[bs]).reshape(BPC * H, S),
            "w1t": w1t_np,
            "vvec": vvec_np,
            "qbt": np.ascontiguousarray(qb_all[bs].T),
        })
    return in_maps


_CACHED_NC = None


def kernel(encoder_outputs, decoder_hidden, W1, b1, W2, b2, V, bv, _trace=False):
    global _CACHED_NC
    from concourse import bass_utils

    if _CACHED_NC is None:
        _CACHED_NC = build_program()
    nc = _CACHED_NC

    in_maps = prepare_in_maps(encoder_outputs, decoder_hidden, W1, b1, W2, b2, V, bv)
    res = bass_utils.run_bass_kernel_spmd(
        nc, in_maps, core_ids=list(range(NCORES)), trace=_trace,
    )
    context = np.concatenate(
        [res.results[c]["ctx_out"] for c in range(NCORES)], axis=0
    ).astype(np.float32)
    attn = np.concatenate(
        [res.results[c]["attn_out"].reshape(BPC, S) for c in range(NCORES)], axis=0
    ).astype(np.float32)
    if _trace:
        kernel._last_results = res
    return context, attn


# revision 29
# speedup vs baseline: 1.2475x; 1.2475x over previous
"""Bahdanau attention TRN2 kernel.

Problem shapes (hardcoded):
    encoder_outputs: [64, 2048, 512] f32
    decoder_hidden:  [64, 512] f32
    W1, W2: [512, 512]; b1, b2: [512]; V: [512]; bv: [1]
Returns (context [64, 512] f32, attn [64, 2048] f32), matching the reference.

Sharding: data-parallel over batch across 8 NeuronCores (8 batches/core).

Per-core algorithm (per batch b):
    scores = tanh(enc @ W1.T + qb) @ V        (qb = b1 + b2 + dec @ W2.T, host-computed;
                                               bv dropped - softmax is shift-invariant)
    attn   = softmax(scores)                  (no max-subtract: |scores| <= sum|V| ~ 12)
    context= attn @ enc

Device layouts:
    enc_t  [h, s] bf16  - transposed enc (host prep), moving operand of the projection matmul
    enc_g  [p, j*512+h] bf16 - enc rows grouped s = p*16 + j, rhs of the context matmul
    scores are produced as PSUM [1, 512] rows at partitions {0,32,64,96} (PE col-tiling),
    reshaped to [128, 16] (s = p*16 + j) via small SBUF->SBUF DMAs.
"""

import numpy as np
import ml_dtypes

import concourse.bass as bass
import concourse.tile as tile
from concourse import bacc, bass_isa, mybir
from concourse._compat import with_exitstack

BF16 = mybir.dt.bfloat16
F32 = mybir.dt.float32
NPBF16 = ml_dtypes.bfloat16

B, S, H = 64, 2048, 512
NCORES = 8
BPC = B // NCORES          # batches per core
ST = 4                     # "supertiles" per batch: 512 s-rows each (vdot col-tile groups)
NC_CHUNK = H // 128        # 4 chunks of 128 along h / o
NJ = S // 128              # 16 s-groups of 128 for the context matmul

AFT = mybir.ActivationFunctionType


@with_exitstack
def _bahdanau_tile_kernel(ctx, tc, enc_g, enc_t, w1t, vvec, qbt, attn_out, ctx_out):
    nc = tc.nc

    const = ctx.enter_context(tc.tile_pool(name="const", bufs=1))
    encn_pool = ctx.enter_context(tc.tile_pool(name="encn", bufs=3))
    enct_pool = ctx.enter_context(tc.tile_pool(name="enct", bufs=8))
    energy_pool = ctx.enter_context(tc.tile_pool(name="energy", bufs=18))
    sm_pool = ctx.enter_context(tc.tile_pool(name="sm", bufs=4))
    pe_psum = ctx.enter_context(tc.tile_pool(name="pe_ps", bufs=2, space="PSUM"))
    sc_psum = ctx.enter_context(tc.tile_pool(name="sc_ps", bufs=2, space="PSUM"))
    small_psum = ctx.enter_context(tc.tile_pool(name="small_ps", bufs=2, space="PSUM"))

    # --- constants ---
    w1t_sb = []   # [128, 512] bf16 per h-chunk, cols = o
    vv_sb = []    # [128, 1] bf16 per o-chunk
    qbt_sb = []   # [128, BPC] f32 per o-chunk
    for c in range(NC_CHUNK):
        wt = const.tile([128, H], BF16, name=f"w1t{c}", tag=f"w1t{c}")
        nc.scalar.dma_start(wt[:], w1t[c * 128:(c + 1) * 128, :])
        w1t_sb.append(wt)
        vv = const.tile([128, 32], BF16, name=f"vv{c}", tag=f"vv{c}")
        nc.scalar.dma_start(vv[:], vvec[c * 128:(c + 1) * 128, :])
        vv_sb.append(vv)
        qb = const.tile([128, BPC], F32, name=f"qbt{c}", tag=f"qbt{c}")
        nc.scalar.dma_start(qb[:], qbt[c * 128:(c + 1) * 128, :])
        qbt_sb.append(qb)


    state = {}  # per-batch tiles handed from one pipeline stage to the next

    def emit_loads(b):
        enct = []
        for c in range(NC_CHUNK):
            et = enct_pool.tile([128, S], BF16, name="enct", tag="enct")
            nc.sync.dma_start(et[:], enc_t[b * H + c * 128:b * H + (c + 1) * 128, :])
            enct.append(et)
        encn = encn_pool.tile([128, NJ * H], BF16, name="encn", tag="encn")
        nc.sync.dma_start(encn[:], enc_g[b * 128:(b + 1) * 128, :])
        state[b] = {"enct": enct, "encn": encn}

    def emit_mm1(b):
        st_ = state[b]
        enct = st_["enct"]
        energies = {}
        for stp in range(ST // 2):
            for oc in range(NC_CHUNK):
                pe = pe_psum.tile([128, 1024], F32, name="pe", tag="pe")
                for st2 in range(2):
                    for hc in range(NC_CHUNK):
                        nc.tensor.matmul(
                            pe[:, st2 * 512:(st2 + 1) * 512],
                            lhsT=w1t_sb[hc][:, oc * 128:(oc + 1) * 128],
                            rhs=enct[hc][:, stp * 1024 + st2 * 512:stp * 1024 + (st2 + 1) * 512],
                            start=(hc == 0),
                            stop=(hc == NC_CHUNK - 1),
                        )
                en = energy_pool.tile([128, 1024], BF16, name="en", tag="en")
                nc.scalar.activation(en[:], pe[:], AFT.Tanh, bias=qbt_sb[oc][:, b:b + 1])
                energies[(stp, oc)] = en
        st_["energies"] = energies

    def emit_vdot(b):
        st_ = state[b]
        energies = st_["energies"]
        # V-dot: scores[st*512 + n] at psum partition 32*st (col groups)
        sc = sc_psum.tile([128, 512], F32, name="sc", tag="sc")
        for oc in range(NC_CHUNK):
            for st in range(ST):
                nc.tensor.matmul(
                    sc[32 * st:32 * st + 32, :],
                    lhsT=vv_sb[oc][:],
                    rhs=energies[(st // 2, oc)][:, (st % 2) * 512:(st % 2 + 1) * 512],
                    start=(oc == 0),
                    stop=(oc == NC_CHUNK - 1),
                    tile_position=(0, 32 * st),
                    skip_group_check=True,
                )
        # drain the whole psum tile in one parallel-partition copy, exp it
        # in row layout (ACT's only dep here is the DVE copy), then reshape
        # exp'd scores to [128, 16] (s = p*16 + j) with one SBUF->SBUF DMA
        sc4 = sm_pool.tile([128, 512], F32, name="sc4", tag="sc4")
        nc.vector.tensor_copy(sc4[:], sc[:])
        exp4 = sm_pool.tile([128, 512], F32, name="exp4", tag="exp4")
        nc.scalar.activation(exp4[:], sc4[:], AFT.Exp)
        expT = sm_pool.tile([128, NJ], F32, name="expT", tag="expT")
        nc.gpsimd.dma_start(expT[:], exp4[0:128:32, :])
        st_["expT"] = expT

    def emit_softmax(b):
        st_ = state[b]
        expT = st_["expT"]
        expTb = sm_pool.tile([128, NJ], BF16, name="expTb", tag="expTb")
        nc.vector.tensor_copy(expTb[:], expT[:])

        expP = sm_pool.tile([128, 1], F32, name="expP", tag="expP")
        nc.vector.tensor_reduce(
            expP[:], expT[:], axis=mybir.AxisListType.X, op=mybir.AluOpType.add
        )
        sum_bc = sm_pool.tile([128, 1], F32, name="sum_bc", tag="sum_bc")
        nc.gpsimd.partition_all_reduce(
            sum_bc[:], expP[:], channels=128, reduce_op=bass_isa.ReduceOp.add
        )
        recip_bc = sm_pool.tile([128, 1], F32, name="recip_bc", tag="recip_bc")
        nc.vector.reciprocal(recip_bc[:], sum_bc[:])
        st_.update(expT=expT, expTb=expTb, recip_bc=recip_bc)

    def emit_mm2_outputs(b):
        st_ = state[b]
        encn, expT, expTb, recip_bc = st_["encn"], st_["expT"], st_["expTb"], st_["recip_bc"]
        ps_ctx = small_psum.tile([1, 512], F32, name="ps_ctx", tag="small")
        for j in range(NJ):
            nc.tensor.matmul(
                ps_ctx[:],
                lhsT=expTb[:, j:j + 1],
                rhs=encn[:, j * 512:(j + 1) * 512],
                start=(j == 0),
                stop=(j == NJ - 1),
            )
        ctx_sb = sm_pool.tile([1, 512], F32, name="ctx_sb", tag="ctx_sb")
        nc.vector.tensor_scalar_mul(ctx_sb[:], ps_ctx[:], recip_bc[0:1, :])
        nc.gpsimd.dma_start(ctx_out[b:b + 1, :], ctx_sb[:])

        attn_sb = sm_pool.tile([128, NJ], F32, name="attn_sb", tag="attn_sb")
        nc.vector.tensor_scalar_mul(attn_sb[:], expT[:], recip_bc[:])
        nc.gpsimd.dma_start(attn_out[b * 128:(b + 1) * 128, :], attn_sb[:])
        del state[b]

    # Software pipeline: PE stream per batch is
    #   ... vdot(b-1), mm1(b), mm2(b-1), vdot(b), mm1(b+1), ...
    # The exp of b-1 is emitted AFTER tanh(b) on the ACT queue so the
    # reshape-DMA wait never head-of-line-blocks the tanh stream.
    for b in range(BPC):
        emit_loads(b)
        if b >= 1:
            emit_vdot(b - 1)
        emit_mm1(b)
        if b >= 1:
            emit_softmax(b - 1)
            emit_mm2_outputs(b - 1)
    emit_vdot(BPC - 1)
    emit_softmax(BPC - 1)
    emit_mm2_outputs(BPC - 1)


def build_program():
    nc = bacc.Bacc(
        "TRN2",
        target_bir_lowering=False,
        debug=False,
        enable_asserts=False,
        num_devices=NCORES,
    )
    enc_g = nc.dram_tensor("enc_g", [BPC * 128, NJ * H], BF16, kind="ExternalInput").ap()
    enc_t = nc.dram_tensor("enc_t", [BPC * H, S], BF16, kind="ExternalInput").ap()
    w1t = nc.dram_tensor("w1t", [H, H], BF16, kind="ExternalInput").ap()
    vvec = nc.dram_tensor("vvec", [H, 32], BF16, kind="ExternalInput").ap()
    qbt = nc.dram_tensor("qbt", [H, BPC], F32, kind="ExternalInput").ap()
    attn_out = nc.dram_tensor("attn_out", [BPC * 128, NJ], F32, kind="ExternalOutput").ap()
    ctx_out = nc.dram_tensor("ctx_out", [BPC, H], F32, kind="ExternalOutput").ap()

    with tile.TileContext(nc) as tc:
        _bahdanau_tile_kernel(tc, enc_g, enc_t, w1t, vvec, qbt, attn_out, ctx_out)
    nc.compile()
    return nc


def prepare_in_maps(encoder_outputs, decoder_hidden, W1, b1, W2, b2, V, bv):
    enc = np.asarray(encoder_outputs, dtype=np.float32)
    dec = np.asarray(decoder_hidden, dtype=np.float32)
    W1 = np.asarray(W1, dtype=np.float32)
    W2 = np.asarray(W2, dtype=np.float32)
    b1 = np.asarray(b1, dtype=np.float32)
    b2 = np.asarray(b2, dtype=np.float32)
    V = np.asarray(V, dtype=np.float32)

    enc_bf = enc.astype(NPBF16)
    # s = p*16 + j grouping: [B, 128, 16, H] -> per core [BPC*128, 16*H]
    enc_g_all = enc_bf.reshape(B, 128, NJ, H)
    enc_t_all = np.ascontiguousarray(enc_bf.transpose(0, 2, 1))  # [B, H, S]

    qb_all = (b1 + b2 + dec @ W2.T).astype(np.float32)  # [B, H]
    w1t_np = np.ascontiguousarray(W1.T).astype(NPBF16)  # [h, o]
    vvec_np = np.ascontiguousarray(np.tile(V.astype(NPBF16).reshape(H, 1), (1, 32)))

    in_maps = []
    for c in range(NCORES):
        bs = slice(c * BPC, (c + 1) * BPC)
        in_maps.append({
            "enc_g": np.ascontiguousarray(enc_g_all[bs]).reshape(BPC * 128, NJ * H),
            "enc_t": np.ascontiguousarray(enc_t_all[bs]).reshape(BPC * H, S),
            "w1t": w1t_np,
            "vvec": vvec_np,
            "qbt": np.ascontiguousarray(qb_all[bs].T),
        })
    return in_maps


_CACHED_NC = None


def kernel(encoder_outputs, decoder_hidden, W1, b1, W2, b2, V, bv, _trace=False):
    global _CACHED_NC
    from concourse import bass_utils

    if _CACHED_NC is None:
        _CACHED_NC = build_program()
    nc = _CACHED_NC

    in_maps = prepare_in_maps(encoder_outputs, decoder_hidden, W1, b1, W2, b2, V, bv)
    res = bass_utils.run_bass_kernel_spmd(
        nc, in_maps, core_ids=list(range(NCORES)), trace=_trace,
    )
    context = np.concatenate(
        [res.results[c]["ctx_out"] for c in range(NCORES)], axis=0
    ).astype(np.float32)
    attn = np.concatenate(
        [res.results[c]["attn_out"].reshape(BPC, S) for c in range(NCORES)], axis=0
    ).astype(np.float32)
    if _trace:
        kernel._last_results = res
    return context, attn
